# revision 15
# baseline (speedup 1.0000x reference)
"""Causal self-attention (B=4, S=2048, D=1024, single head) on 8 TRN2 cores.

Sharding: data-parallel over batch (4 batches x 2 cores), with the two cores
of a batch splitting the KEY dimension (core par=0 takes even key tiles,
par=1 odd). Each core projects Q for ALL 2048 queries of its batch but K/V
only for its 8 key tiles, computes scores TRANSPOSED (S^T tiles [key, query]
directly off PE — no PE transposes needed), exponentiates into a packed P^T
buffer, and accumulates an UNNORMALIZED numerator N = P^T.T @ V plus the
denominator l = colsum(P^T). The host combines the two partials per batch:
out = (N_even + N_odd) / (l_even + l_odd).

vs the query-split baseline this trades the duplicated K+V projection
(2x 4.3 GFLOP/core) for a duplicated Q projection (2.1 GFLOP/core) and
removes all 72 PE transposes per core.

Per key tile jl (global tile j = 2*jl+par), scores are computed against the
query range [2*jl*128, 2048) — one program for both cores; the first 256
query columns carry a host-built additive mask (triangular diagonal for the
variant where j is the range start, full -60 + triangular for the other).

Input DMAs are coalesced (one trigger moves all 8 contraction strips via a
3D access pattern) because DMA_DIRECT2D triggers cost ~0.6us each on the
sync engine and fine-grained tile DMAs cap effective inbound bandwidth.
"""

import os
from contextlib import ExitStack

import ml_dtypes
import numpy as np

import concourse.bacc as bacc
import concourse.mybir as mybir
import concourse.tile as tile
from concourse.bass_utils import run_bass_kernel_spmd

B, S, D = 4, 2048, 1024
P = 128
DC = D // P  # 8 contraction chunks
NKT = 8      # own key tiles per core
MASK_VAL = -60.0

F32 = mybir.dt.float32
F16 = mybir.dt.float16
BF16 = mybir.dt.bfloat16

W_JL = [(16 - 2 * jl) * P for jl in range(NKT)]  # 2048,1792,...,256
OFF_JL = [0]
for _w in W_JL[:-1]:
    OFF_JL.append(OFF_JL[-1] + _w)
PT_COLS = OFF_JL[-1] + W_JL[-1]  # 9216

_compiled = {}


def _strips(dram_2d_slice):
    """View a [D, C] DRAM slice as [128, DC, C] (partition-major strips)."""
    return dram_2d_slice.rearrange("(d p) c -> p d c", p=P)


def _build():
    nc = bacc.Bacc("TRN2", target_bir_lowering=False, debug=False)
    xqh = nc.dram_tensor("xqh", [D, S // 2], BF16, kind="ExternalInput").ap()  # own half of x^T/32
    xkT = nc.dram_tensor("xkT", [D, NKT * P], BF16, kind="ExternalInput").ap()
    wq = nc.dram_tensor("wq", [D, D], BF16, kind="ExternalInput").ap()
    wk = nc.dram_tensor("wk", [D, D], BF16, kind="ExternalInput").ap()
    wv = nc.dram_tensor("wv", [D, D], BF16, kind="ExternalInput").ap()
    msk = nc.dram_tensor("msk", [NKT * P, 2 * P], F32, kind="ExternalInput").ap()
    out_d = nc.dram_tensor("out", [S, D], F16, kind="ExternalOutput").ap()
    lout = nc.dram_tensor("lout", [1, S], F32, kind="ExternalOutput").ap()
    with tile.TileContext(nc) as tc:
        _body(tc, xqh, xkT, wq, wk, wv, msk, out_d, lout)
    nc.compile()
    return nc


def _body(tc, xqh, xkT, wq, wk, wv, msk, out_d, lout):
    nc = tc.nc
    with ExitStack() as top:
        const_pool = top.enter_context(tc.tile_pool(name="cst", bufs=1))
        ones = const_pool.tile([P, 1], BF16, name="ones", tag="ones")
        nc.gpsimd.memset(ones[:], 1.0)

        res = top.enter_context(tc.tile_pool(name="res", bufs=1))
        QT = [res.tile([P, S], BF16, name=f"qt{e}", tag=f"qt{e}") for e in range(DC)]
        KT = [res.tile([P, NKT * P], BF16, name=f"kt{e}", tag=f"kt{e}") for e in range(DC)]
        V = [res.tile([P, D], BF16, name=f"v{j}", tag=f"v{j}") for j in range(NKT)]
        PT = res.tile([P, PT_COLS], BF16, name="ptb", tag="ptb")
        lsb = res.tile([1, S], F32, name="lsb", tag="lsb")
        mskb = res.tile([P, NKT * 2 * P], F32, name="mskb", tag="mskb")

        # DRAM staging for the intra-pair Q^T AllGather
        dram = top.enter_context(tc.tile_pool(name="ccd", bufs=1, space="DRAM"))
        g_in = dram.tile([D, S // 2], BF16, name="gin", tag="gin")
        g_out = dram.tile([2 * D, S // 2], BF16, name="gout", tag="gout")

        # ---------------- projections: V (first — smallest DMA gate), K, Q ----
        with ExitStack() as ph:
            # Input DMAs take ~9us to deliver first bytes; keep the PE busy
            # (and the HAM clock-gate warm) with throwaway matmuls meanwhile.
            wup = ph.enter_context(tc.tile_pool(name="wup", bufs=2, space="PSUM"))
            wsrc = const_pool.tile([P, 512], BF16, name="wsrc", tag="wsrc")
            nc.gpsimd.memset(wsrc[:], 0.5)
            wdst = const_pool.tile([P, 512], F32, name="wdst", tag="wdst")
            for i in range(32):
                wps = wup.tile([P, 512], F32, name="wps", tag="wu")
                nc.tensor.matmul(
                    wps[:], lhsT=wsrc[:, 0:P], rhs=wsrc[:], start=True, stop=True
                )
                if i >= 30:
                    nc.vector.tensor_copy(wdst[:], wps[:])
            wvp = ph.enter_context(tc.tile_pool(name="wvp", bufs=1))
            wkp = ph.enter_context(tc.tile_pool(name="wkp", bufs=1))
            wqp = ph.enter_context(tc.tile_pool(name="wqp", bufs=1))
            xkp = ph.enter_context(tc.tile_pool(name="xkp", bufs=1))
            xqp = ph.enter_context(tc.tile_pool(name="xqp", bufs=1))
            pps = ph.enter_context(tc.tile_pool(name="pps", bufs=3, space="PSUM"))
            # strip layout: column d*W + c holds (d-th contraction strip, col c)
            wv_lo = wvp.tile([P, DC * 512], BF16, name="wvlo", tag="wvlo")
            wv_hi = wvp.tile([P, DC * 512], BF16, name="wvhi", tag="wvhi")
            xk_lo = xkp.tile([P, DC * 512], BF16, name="xklo", tag="xklo")
            xk_hi = xkp.tile([P, DC * 512], BF16, name="xkhi", tag="xkhi")
            wk_b = wkp.tile([P, DC * D], BF16, name="wkb", tag="wkb")
            wq_b = wqp.tile([P, DC * D], BF16, name="wqb", tag="wqb")
            xq_c = [xqp.tile([P, DC * 512], BF16, name=f"xq{qc}", tag=f"xq{qc}") for qc in range(2)]
            qth = xqp.tile([P, DC * (S // 2)], BF16, name="qth", tag="qth")

            def strip3(t, w):
                return t[:].rearrange("p (d c) -> p d c", d=DC)

            # one coalesced trigger per tensor(-half), in dependency order
            nc.sync.dma_start(strip3(xk_lo, 512), _strips(xkT[:, 0:512]))
            nc.sync.dma_start(strip3(wv_lo, 512), _strips(wv[:, 0:512]))
            nc.sync.dma_start(strip3(xk_hi, 512), _strips(xkT[:, 512:1024]))
            nc.sync.dma_start(strip3(wv_hi, 512), _strips(wv[:, 512:1024]))
            nc.sync.dma_start(strip3(wq_b, D), _strips(wq[:, :]))
            for qc in range(2):
                nc.sync.dma_start(
                    strip3(xq_c[qc], 512), _strips(xqh[:, qc * 512 : (qc + 1) * 512])
                )
            nc.sync.dma_start(strip3(wk_b, D), _strips(wk[:, :]))
            nc.sync.dma_start(
                mskb[:].rearrange("p (j c) -> p j c", j=NKT),
                msk.rearrange("(j p) c -> p j c", p=P),
            )

            def xk_sl(d, jl):
                t, o = (xk_lo, jl) if jl < 4 else (xk_hi, jl - 4)
                return t[:, d * 512 + o * P : d * 512 + (o + 1) * P]

            # V[jl] = x_j @ Wv  (x stationary, Wv moving); ec-outer so the
            # second halves of wv/xk have time to land
            for ec in range(2):
                wvt = wv_lo if ec == 0 else wv_hi
                for jl in range(NKT):
                    ps = pps.tile([P, 512], F32, name="pps", tag="pps")
                    for d in range(DC):
                        nc.tensor.matmul(
                            ps[:],
                            lhsT=xk_sl(d, jl),
                            rhs=wvt[:, d * 512 : (d + 1) * 512],
                            start=(d == 0),
                            stop=(d == DC - 1),
                        )
                    nc.scalar.copy(V[jl][:, ec * 512 : (ec + 1) * 512], ps[:])

            # Q^T of OWN query half (e-major; x pre-scaled by 1/32), staged to
            # DRAM and AllGathered within the pair while K^T projects.
            for e in range(DC):
                for qc in range(2):
                    ps = pps.tile([P, 512], F32, name="pps", tag="pps")
                    for d in range(DC):
                        nc.tensor.matmul(
                            ps[:],
                            lhsT=wq_b[:, d * D + e * P : d * D + (e + 1) * P],
                            rhs=xq_c[qc][:, d * 512 : (d + 1) * 512],
                            start=(d == 0),
                            stop=(d == DC - 1),
                        )
                    nc.scalar.copy(
                        qth[:, e * (S // 2) + qc * 512 : e * (S // 2) + (qc + 1) * 512],
                        ps[:],
                    )
            nc.sync.dma_start(
                g_in[:, :].rearrange("(e p) q -> p e q", p=P),
                qth[:].rearrange("p (e q) -> p e q", e=DC),
            )
            nc.gpsimd.collective_compute(
                "AllGather",
                mybir.AluOpType.bypass,
                replica_groups=[[0, 1], [2, 3], [4, 5], [6, 7]],
                ins=[g_in[:, :]],
                outs=[g_out[:, :]],
            )

            # K^T (e-major over own keys)
            for e in range(DC):
                for kc in range(2):
                    xkt = xk_lo if kc == 0 else xk_hi
                    ps = pps.tile([P, 512], F32, name="pps", tag="pps")
                    for d in range(DC):
                        nc.tensor.matmul(
                            ps[:],
                            lhsT=wk_b[:, d * D + e * P : d * D + (e + 1) * P],
                            rhs=xkt[:, d * 512 : (d + 1) * 512],
                            start=(d == 0),
                            stop=(d == DC - 1),
                        )
                    nc.scalar.copy(KT[e][:, kc * 512 : (kc + 1) * 512], ps[:])

            # pull the gathered full Q^T back into SBUF (both halves)
            for e in range(DC):
                nc.sync.dma_start(
                    QT[e][:, 0 : S // 2], g_out[e * P : (e + 1) * P, :]
                )
                nc.sync.dma_start(
                    QT[e][:, S // 2 : S], g_out[D + e * P : D + (e + 1) * P, :]
                )

        # ---------------- attention ----------------
        with ExitStack() as ph:
            ob = ph.enter_context(tc.tile_pool(name="ob", bufs=6))
            qkp = ph.enter_context(tc.tile_pool(name="qkp", bufs=2, space="PSUM"))
            lp = ph.enter_context(tc.tile_pool(name="lp", bufs=2, space="PSUM"))
            opp = ph.enter_context(tc.tile_pool(name="opp", bufs=4, space="PSUM"))

            # Interleaved rounds: after key tile jl's scores+exp are done, the
            # numerators for query tiles 2jl and 2jl+1 are complete (they need
            # exp tiles jl' <= jl only) — emit their PV immediately so output
            # DMA streams throughout the phase instead of draining at the end.
            # The denominator chunk qc completes after jl = 2qc+1 likewise.
            for jl in range(NKT):
                # scores^T + exp for own key tile jl
                Wj = W_JL[jl]
                qb = 2 * jl * P
                nch = (Wj + 511) // 512
                for c in range(nch):
                    nw = min(512, Wj - c * 512)
                    ps = qkp.tile([P, 512], F32, name="qk", tag="qk")
                    for e in range(DC):
                        nc.tensor.matmul(
                            ps[:, :nw],
                            lhsT=KT[e][:, jl * P : (jl + 1) * P],
                            rhs=QT[e][:, qb + c * 512 : qb + c * 512 + nw],
                            start=(e == 0),
                            stop=(e == DC - 1),
                        )
                    if c == 0:
                        nc.vector.tensor_add(
                            ps[:, 0 : 2 * P],
                            ps[:, 0 : 2 * P],
                            mskb[:, jl * 2 * P : (jl + 1) * 2 * P],
                        )
                    nc.scalar.activation(
                        PT[:, OFF_JL[jl] + c * 512 : OFF_JL[jl] + c * 512 + nw],
                        ps[:, :nw],
                        mybir.ActivationFunctionType.Exp,
                    )

                # denominator chunk: l[q] = sum_k P^T[k, q], ones-stationary
                if jl % 2 == 1:
                    qc = (jl - 1) // 2
                    q0, q1 = qc * 512, qc * 512 + 512
                    js = [j2 for j2 in range(NKT) if 2 * j2 * P < q1]
                    lps = lp.tile([1, 512], F32, name="lps", tag="lps")
                    for i, j2 in enumerate(js):
                        s = max(q0, 2 * j2 * P)
                        w = q1 - s
                        o = OFF_JL[j2] + (s - 2 * j2 * P)
                        nc.tensor.matmul(
                            lps[0:1, s - q0 : 512],
                            lhsT=ones[:],
                            rhs=PT[:, o : o + w],
                            start=(i == 0),
                            stop=(i == len(js) - 1),
                            skip_group_check=True,
                        )
                    nc.scalar.copy(lsb[0:1, q0:q1], lps[:])
                    if qc == 3:
                        nc.scalar.dma_start(lout[0:1, :], lsb[:])

                # numerators for query tiles 2jl, 2jl+1
                for t in (2 * jl, 2 * jl + 1):
                    njl = t // 2 + 1
                    ot = ob.tile([P, D], F16, name="ot", tag="ot")
                    for ec in range(2):
                        ops = opp.tile([P, 512], F32, name="ops", tag="ops")
                        for j2 in range(njl):
                            o = OFF_JL[j2] + (t - 2 * j2) * P
                            nc.tensor.matmul(
                                ops[:],
                                lhsT=PT[:, o : o + P],
                                rhs=V[j2][:, ec * 512 : (ec + 1) * 512],
                                start=(j2 == 0),
                                stop=(j2 == njl - 1),
                            )
                        if ec == 0:
                            nc.scalar.copy(ot[:, 0:512], ops[:])
                        else:
                            nc.vector.tensor_copy(ot[:, 512:1024], ops[:])
                    # out-DMA on the Activation HWDGE queue — separate ring
                    # from the input/readback traffic on sync
                    nc.scalar.dma_start(out_d[t * P : (t + 1) * P, :], ot[:])


def _get_nc():
    if "nc" not in _compiled:
        _compiled["nc"] = _build()
    return _compiled["nc"]


def kernel(x, Wq, Wk, Wv):
    x = np.ascontiguousarray(np.asarray(x, dtype=np.float32))
    Wq = np.asarray(Wq, dtype=np.float32)
    Wk = np.asarray(Wk, dtype=np.float32)
    Wv = np.asarray(Wv, dtype=np.float32)

    nc = _get_nc()
    bf16 = ml_dtypes.bfloat16
    Wq_c = np.ascontiguousarray(Wq.astype(bf16))
    Wk_c = np.ascontiguousarray(Wk.astype(bf16))
    Wv_c = np.ascontiguousarray(Wv.astype(bf16))

    in_maps = []
    for c in range(8):
        b, par = c // 2, c % 2
        xb = x[b]  # [S, D]
        xqh_np = np.ascontiguousarray(
            (xb[par * (S // 2) : (par + 1) * (S // 2)].T * np.float32(1.0 / 32.0)).astype(bf16)
        )
        keys = np.concatenate(
            [np.arange((2 * i + par) * P, (2 * i + par + 1) * P) for i in range(NKT)]
        )
        xkT_np = np.ascontiguousarray(xb.T[:, keys].astype(bf16))
        m = np.empty((NKT * P, 2 * P), np.float32)
        for jl in range(NKT):
            j = 2 * jl + par
            kglob = np.arange(j * P, (j + 1) * P)
            qglob = np.arange(2 * jl * P, 2 * jl * P + 2 * P)
            m[jl * P : (jl + 1) * P, :] = np.where(
                qglob[None, :] >= kglob[:, None], np.float32(0.0), np.float32(MASK_VAL)
            )
        in_maps.append(
            {
                "xqh": xqh_np,
                "xkT": xkT_np,
                "wq": Wq_c,
                "wk": Wk_c,
                "wv": Wv_c,
                "msk": np.ascontiguousarray(m),
            }
        )

    trace = os.environ.get("BASS_KERNEL_TRACE", "0") == "1"
    res = run_bass_kernel_spmd(nc, in_maps, core_ids=list(range(8)), trace=trace)
    if trace:
        print(f"HW exec time: {res.exec_time_ns} ns")
        if res.instructions_and_trace is not None:
            print(f"trace: {res.instructions_and_trace[1]}")

    out = np.empty((B, S, D), np.float32)
    for b in range(B):
        n0 = res.results[2 * b]["out"].astype(np.float32)
        n1 = res.results[2 * b + 1]["out"].astype(np.float32)
        l0 = res.results[2 * b]["lout"][0].astype(np.float32)
        l1 = res.results[2 * b + 1]["lout"][0].astype(np.float32)
        out[b] = (n0 + n1) / (l0 + l1)[:, None]
    return out


# revision 17
# speedup vs baseline: 1.1244x; 1.1244x over previous
"""Causal self-attention (B=4, S=2048, D=1024, single head) on 8 TRN2 cores.

Sharding: data-parallel over batch (4 batches x 2 cores), with the two cores
of a batch splitting the KEY dimension (core par=0 takes even key tiles,
par=1 odd). Each core projects Q for ALL 2048 queries of its batch but K/V
only for its 8 key tiles, computes scores TRANSPOSED (S^T tiles [key, query]
directly off PE — no PE transposes needed), exponentiates into a packed P^T
buffer, and accumulates an UNNORMALIZED numerator N = P^T.T @ V plus the
denominator l = colsum(P^T). The host combines the two partials per batch:
out = (N_even + N_odd) / (l_even + l_odd).

vs the query-split baseline this trades the duplicated K+V projection
(2x 4.3 GFLOP/core) for a duplicated Q projection (2.1 GFLOP/core) and
removes all 72 PE transposes per core.

Per key tile jl (global tile j = 2*jl+par), scores are computed against the
query range [2*jl*128, 2048) — one program for both cores; the first 256
query columns carry a host-built additive mask (triangular diagonal for the
variant where j is the range start, full -60 + triangular for the other).

Input DMAs are coalesced (one trigger moves all 8 contraction strips via a
3D access pattern) because DMA_DIRECT2D triggers cost ~0.6us each on the
sync engine and fine-grained tile DMAs cap effective inbound bandwidth.
"""

import os
from contextlib import ExitStack

import ml_dtypes
import numpy as np

import concourse.bacc as bacc
import concourse.mybir as mybir
import concourse.tile as tile
from concourse.bass_utils import run_bass_kernel_spmd

B, S, D = 4, 2048, 1024
P = 128
DC = D // P  # 8 contraction chunks
NKT = 8      # own key tiles per core
MASK_VAL = -60.0

F32 = mybir.dt.float32
F16 = mybir.dt.float16
BF16 = mybir.dt.bfloat16

W_JL = [(16 - 2 * jl) * P for jl in range(NKT)]  # 2048,1792,...,256
OFF_JL = [0]
for _w in W_JL[:-1]:
    OFF_JL.append(OFF_JL[-1] + _w)
PT_COLS = OFF_JL[-1] + W_JL[-1]  # 9216

_compiled = {}


def _strips(dram_2d_slice):
    """View a [D, C] DRAM slice as [128, DC, C] (partition-major strips)."""
    return dram_2d_slice.rearrange("(d p) c -> p d c", p=P)


def _build():
    nc = bacc.Bacc("TRN2", target_bir_lowering=False, debug=False)
    xqh = nc.dram_tensor("xqh", [D, S // 2], BF16, kind="ExternalInput").ap()  # own half of x^T/32
    xkT = nc.dram_tensor("xkT", [D, NKT * P], BF16, kind="ExternalInput").ap()
    wq = nc.dram_tensor("wq", [D, D], BF16, kind="ExternalInput").ap()
    wk = nc.dram_tensor("wk", [D, D], BF16, kind="ExternalInput").ap()
    wv = nc.dram_tensor("wv", [D, D], BF16, kind="ExternalInput").ap()
    msk = nc.dram_tensor("msk", [NKT * P, 2 * P], F32, kind="ExternalInput").ap()
    out_d = nc.dram_tensor("out", [S, D], F16, kind="ExternalOutput").ap()
    lout = nc.dram_tensor("lout", [1, S], F32, kind="ExternalOutput").ap()
    with tile.TileContext(nc) as tc:
        _body(tc, xqh, xkT, wq, wk, wv, msk, out_d, lout)
    nc.compile()
    return nc


def _body(tc, xqh, xkT, wq, wk, wv, msk, out_d, lout):
    nc = tc.nc
    with ExitStack() as top:
        const_pool = top.enter_context(tc.tile_pool(name="cst", bufs=1))
        ones = const_pool.tile([P, 1], BF16, name="ones", tag="ones")
        nc.gpsimd.memset(ones[:], 1.0)

        res = top.enter_context(tc.tile_pool(name="res", bufs=1))
        QT = [res.tile([P, S], BF16, name=f"qt{e}", tag=f"qt{e}") for e in range(DC)]
        KT = [res.tile([P, NKT * P], BF16, name=f"kt{e}", tag=f"kt{e}") for e in range(DC)]
        V = [res.tile([P, D], BF16, name=f"v{j}", tag=f"v{j}") for j in range(NKT)]
        PT = res.tile([P, PT_COLS], BF16, name="ptb", tag="ptb")
        lsb = res.tile([1, S], F32, name="lsb", tag="lsb")
        mskb = res.tile([P, NKT * 2 * P], F32, name="mskb", tag="mskb")

        # DRAM staging for the intra-pair Q^T AllGather
        dram = top.enter_context(tc.tile_pool(name="ccd", bufs=1, space="DRAM"))
        g_in = dram.tile([D, S // 2], BF16, name="gin", tag="gin")
        g_out = dram.tile([2 * D, S // 2], BF16, name="gout", tag="gout")

        # ---------------- projections: V (first — smallest DMA gate), K, Q ----
        with ExitStack() as ph:
            # Input DMAs take ~9us to deliver first bytes; keep the PE busy
            # (and the HAM clock-gate warm) with throwaway matmuls meanwhile.
            wup = ph.enter_context(tc.tile_pool(name="wup", bufs=2, space="PSUM"))
            wsrc = const_pool.tile([P, 512], BF16, name="wsrc", tag="wsrc")
            nc.gpsimd.memset(wsrc[:], 0.5)
            wdst = const_pool.tile([P, 512], F32, name="wdst", tag="wdst")
            for i in range(32):
                wps = wup.tile([P, 512], F32, name="wps", tag="wu")
                nc.tensor.matmul(
                    wps[:], lhsT=wsrc[:, 0:P], rhs=wsrc[:], start=True, stop=True
                )
                if i >= 30:
                    nc.vector.tensor_copy(wdst[:], wps[:])
            wvp = ph.enter_context(tc.tile_pool(name="wvp", bufs=1))
            wkp = ph.enter_context(tc.tile_pool(name="wkp", bufs=1))
            wqp = ph.enter_context(tc.tile_pool(name="wqp", bufs=1))
            xkp = ph.enter_context(tc.tile_pool(name="xkp", bufs=1))
            xqp = ph.enter_context(tc.tile_pool(name="xqp", bufs=1))
            pps = ph.enter_context(tc.tile_pool(name="pps", bufs=3, space="PSUM"))
            # strip layout: column d*W + c holds (d-th contraction strip, col c)
            wv_lo = wvp.tile([P, DC * 512], BF16, name="wvlo", tag="wvlo")
            wv_hi = wvp.tile([P, DC * 512], BF16, name="wvhi", tag="wvhi")
            xk_lo = xkp.tile([P, DC * 512], BF16, name="xklo", tag="xklo")
            xk_hi = xkp.tile([P, DC * 512], BF16, name="xkhi", tag="xkhi")
            wk_b = wkp.tile([P, DC * D], BF16, name="wkb", tag="wkb")
            wq_b = wqp.tile([P, DC * D], BF16, name="wqb", tag="wqb")
            xq_c = [xqp.tile([P, DC * 512], BF16, name=f"xq{qc}", tag=f"xq{qc}") for qc in range(2)]
            qth = xqp.tile([P, DC * (S // 2)], BF16, name="qth", tag="qth")

            def strip3(t, w):
                return t[:].rearrange("p (d c) -> p d c", d=DC)

            # one coalesced trigger per tensor(-half), in dependency order:
            # Q-half runs first so its AllGather hides under V+K projection
            nc.sync.dma_start(strip3(wq_b, D), _strips(wq[:, :]))
            for qc in range(2):
                nc.sync.dma_start(
                    strip3(xq_c[qc], 512), _strips(xqh[:, qc * 512 : (qc + 1) * 512])
                )
            nc.sync.dma_start(strip3(xk_lo, 512), _strips(xkT[:, 0:512]))
            nc.sync.dma_start(strip3(wv_lo, 512), _strips(wv[:, 0:512]))
            nc.sync.dma_start(strip3(xk_hi, 512), _strips(xkT[:, 512:1024]))
            nc.sync.dma_start(strip3(wv_hi, 512), _strips(wv[:, 512:1024]))
            nc.sync.dma_start(strip3(wk_b, D), _strips(wk[:, :]))
            nc.sync.dma_start(
                mskb[:].rearrange("p (j c) -> p j c", j=NKT),
                msk.rearrange("(j p) c -> p j c", p=P),
            )

            def xk_sl(d, jl):
                t, o = (xk_lo, jl) if jl < 4 else (xk_hi, jl - 4)
                return t[:, d * 512 + o * P : d * 512 + (o + 1) * P]

            # Q^T of OWN query half (e-major; x pre-scaled by 1/32), staged to
            # DRAM and AllGathered within the pair while K^T projects.
            for e in range(DC):
                for qc in range(2):
                    ps = pps.tile([P, 512], F32, name="pps", tag="pps")
                    for d in range(DC):
                        nc.tensor.matmul(
                            ps[:],
                            lhsT=wq_b[:, d * D + e * P : d * D + (e + 1) * P],
                            rhs=xq_c[qc][:, d * 512 : (d + 1) * 512],
                            start=(d == 0),
                            stop=(d == DC - 1),
                        )
                    nc.scalar.copy(
                        qth[:, e * (S // 2) + qc * 512 : e * (S // 2) + (qc + 1) * 512],
                        ps[:],
                    )
            nc.sync.dma_start(
                g_in[:, :].rearrange("(e p) q -> p e q", p=P),
                qth[:].rearrange("p (e q) -> p e q", e=DC),
            )
            nc.gpsimd.collective_compute(
                "AllGather",
                mybir.AluOpType.bypass,
                replica_groups=[[0, 1], [2, 3], [4, 5], [6, 7]],
                ins=[g_in[:, :]],
                outs=[g_out[:, :]],
            )

            # V[jl] = x_j @ Wv  (x stationary, Wv moving); ec-outer so the
            # second halves of wv/xk have time to land
            for ec in range(2):
                wvt = wv_lo if ec == 0 else wv_hi
                for jl in range(NKT):
                    ps = pps.tile([P, 512], F32, name="pps", tag="pps")
                    for d in range(DC):
                        nc.tensor.matmul(
                            ps[:],
                            lhsT=xk_sl(d, jl),
                            rhs=wvt[:, d * 512 : (d + 1) * 512],
                            start=(d == 0),
                            stop=(d == DC - 1),
                        )
                    nc.scalar.copy(V[jl][:, ec * 512 : (ec + 1) * 512], ps[:])

            # K^T (e-major over own keys)
            for e in range(DC):
                for kc in range(2):
                    xkt = xk_lo if kc == 0 else xk_hi
                    ps = pps.tile([P, 512], F32, name="pps", tag="pps")
                    for d in range(DC):
                        nc.tensor.matmul(
                            ps[:],
                            lhsT=wk_b[:, d * D + e * P : d * D + (e + 1) * P],
                            rhs=xkt[:, d * 512 : (d + 1) * 512],
                            start=(d == 0),
                            stop=(d == DC - 1),
                        )
                    nc.scalar.copy(KT[e][:, kc * 512 : (kc + 1) * 512], ps[:])

            # pull the gathered full Q^T back into SBUF (both halves)
            for e in range(DC):
                nc.sync.dma_start(
                    QT[e][:, 0 : S // 2], g_out[e * P : (e + 1) * P, :]
                )
                nc.sync.dma_start(
                    QT[e][:, S // 2 : S], g_out[D + e * P : D + (e + 1) * P, :]
                )

        # ---------------- attention ----------------
        with ExitStack() as ph:
            ob = ph.enter_context(tc.tile_pool(name="ob", bufs=6))
            qkp = ph.enter_context(tc.tile_pool(name="qkp", bufs=2, space="PSUM"))
            lp = ph.enter_context(tc.tile_pool(name="lp", bufs=2, space="PSUM"))
            opp = ph.enter_context(tc.tile_pool(name="opp", bufs=4, space="PSUM"))

            # Interleaved rounds: after key tile jl's scores+exp are done, the
            # numerators for query tiles 2jl and 2jl+1 are complete (they need
            # exp tiles jl' <= jl only) — emit their PV immediately so output
            # DMA streams throughout the phase instead of draining at the end.
            # The denominator chunk qc completes after jl = 2qc+1 likewise.
            for jl in range(NKT):
                # scores^T + exp for own key tile jl
                Wj = W_JL[jl]
                qb = 2 * jl * P
                nch = (Wj + 511) // 512
                for c in range(nch):
                    nw = min(512, Wj - c * 512)
                    ps = qkp.tile([P, 512], F32, name="qk", tag="qk")
                    for e in range(DC):
                        nc.tensor.matmul(
                            ps[:, :nw],
                            lhsT=KT[e][:, jl * P : (jl + 1) * P],
                            rhs=QT[e][:, qb + c * 512 : qb + c * 512 + nw],
                            start=(e == 0),
                            stop=(e == DC - 1),
                        )
                    if c == 0:
                        nc.vector.tensor_add(
                            ps[:, 0 : 2 * P],
                            ps[:, 0 : 2 * P],
                            mskb[:, jl * 2 * P : (jl + 1) * 2 * P],
                        )
                    nc.scalar.activation(
                        PT[:, OFF_JL[jl] + c * 512 : OFF_JL[jl] + c * 512 + nw],
                        ps[:, :nw],
                        mybir.ActivationFunctionType.Exp,
                    )

                # denominator chunk: l[q] = sum_k P^T[k, q], ones-stationary
                if jl % 2 == 1:
                    qc = (jl - 1) // 2
                    q0, q1 = qc * 512, qc * 512 + 512
                    js = [j2 for j2 in range(NKT) if 2 * j2 * P < q1]
                    lps = lp.tile([1, 512], F32, name="lps", tag="lps")
                    for i, j2 in enumerate(js):
                        s = max(q0, 2 * j2 * P)
                        w = q1 - s
                        o = OFF_JL[j2] + (s - 2 * j2 * P)
                        nc.tensor.matmul(
                            lps[0:1, s - q0 : 512],
                            lhsT=ones[:],
                            rhs=PT[:, o : o + w],
                            start=(i == 0),
                            stop=(i == len(js) - 1),
                            skip_group_check=True,
                        )
                    nc.scalar.copy(lsb[0:1, q0:q1], lps[:])
                    if qc == 3:
                        nc.scalar.dma_start(lout[0:1, :], lsb[:])

                # numerators for query tiles 2jl, 2jl+1
                for t in (2 * jl, 2 * jl + 1):
                    njl = t // 2 + 1
                    ot = ob.tile([P, D], F16, name="ot", tag="ot")
                    for ec in range(2):
                        ops = opp.tile([P, 512], F32, name="ops", tag="ops")
                        for j2 in range(njl):
                            o = OFF_JL[j2] + (t - 2 * j2) * P
                            nc.tensor.matmul(
                                ops[:],
                                lhsT=PT[:, o : o + P],
                                rhs=V[j2][:, ec * 512 : (ec + 1) * 512],
                                start=(j2 == 0),
                                stop=(j2 == njl - 1),
                            )
                        if ec == 0:
                            nc.scalar.copy(ot[:, 0:512], ops[:])
                        else:
                            nc.vector.tensor_copy(ot[:, 512:1024], ops[:])
                    # out-DMA on the Activation HWDGE queue — separate ring
                    # from the input/readback traffic on sync
                    nc.scalar.dma_start(out_d[t * P : (t + 1) * P, :], ot[:])


def _get_nc():
    if "nc" not in _compiled:
        _compiled["nc"] = _build()
    return _compiled["nc"]


def kernel(x, Wq, Wk, Wv):
    x = np.ascontiguousarray(np.asarray(x, dtype=np.float32))
    Wq = np.asarray(Wq, dtype=np.float32)
    Wk = np.asarray(Wk, dtype=np.float32)
    Wv = np.asarray(Wv, dtype=np.float32)

    nc = _get_nc()
    bf16 = ml_dtypes.bfloat16
    Wq_c = np.ascontiguousarray(Wq.astype(bf16))
    Wk_c = np.ascontiguousarray(Wk.astype(bf16))
    Wv_c = np.ascontiguousarray(Wv.astype(bf16))

    in_maps = []
    for c in range(8):
        b, par = c // 2, c % 2
        xb = x[b]  # [S, D]
        xqh_np = np.ascontiguousarray(
            (xb[par * (S // 2) : (par + 1) * (S // 2)].T * np.float32(1.0 / 32.0)).astype(bf16)
        )
        keys = np.concatenate(
            [np.arange((2 * i + par) * P, (2 * i + par + 1) * P) for i in range(NKT)]
        )
        xkT_np = np.ascontiguousarray(xb.T[:, keys].astype(bf16))
        m = np.empty((NKT * P, 2 * P), np.float32)
        for jl in range(NKT):
            j = 2 * jl + par
            kglob = np.arange(j * P, (j + 1) * P)
            qglob = np.arange(2 * jl * P, 2 * jl * P + 2 * P)
            m[jl * P : (jl + 1) * P, :] = np.where(
                qglob[None, :] >= kglob[:, None], np.float32(0.0), np.float32(MASK_VAL)
            )
        in_maps.append(
            {
                "xqh": xqh_np,
                "xkT": xkT_np,
                "wq": Wq_c,
                "wk": Wk_c,
                "wv": Wv_c,
                "msk": np.ascontiguousarray(m),
            }
        )

    trace = os.environ.get("BASS_KERNEL_TRACE", "0") == "1"
    res = run_bass_kernel_spmd(nc, in_maps, core_ids=list(range(8)), trace=trace)
    if trace:
        print(f"HW exec time: {res.exec_time_ns} ns")
        if res.instructions_and_trace is not None:
            print(f"trace: {res.instructions_and_trace[1]}")

    out = np.empty((B, S, D), np.float32)
    for b in range(B):
        n0 = res.results[2 * b]["out"].astype(np.float32)
        n1 = res.results[2 * b + 1]["out"].astype(np.float32)
        l0 = res.results[2 * b]["lout"][0].astype(np.float32)
        l1 = res.results[2 * b + 1]["lout"][0].astype(np.float32)
        out[b] = (n0 + n1) / (l0 + l1)[:, None]
    return out
